# revision 3
# baseline (speedup 1.0000x reference)
"""Central-difference L1 loss kernel for 8 trn2 NeuronCores.

Math: with d = x - y, the loss is
    mean_{27 offsets o} |d[v] - d_pad[v + o]|
over the (B,C,D,H,W) = (2,1,32,128,128) volume, zero-padded by 1 in D/H/W.
(The x_diff - y_diff of the reference collapses to differences of d.)

Sharding: 8 shards over (B=2) x (D in 4 chunks of 8 slices). Each core gets a
[128(H), 10(slices incl halo), 130(W incl pad)] fp32 slab of x and y with
zeros in the halo/pad positions that fall outside the volume.

Device per core:
  d = xs - ys                              (DVE subtract)
  dm[h] = d[h-1], dp[h] = d[h+1]           (SBUF->SBUF DMA partition shift)
  for each of 26 nonzero offsets:          (center offset contributes 0)
      acc[2o]   = sum over owned voxels of max(center, shifted)
      acc[2o+1] = sum over owned voxels of min(center, shifted)
  (sum|a-b| = sum max - sum min; DVE tensor_tensor_reduce has no abs op)
Host folds the 8x[128,52] partial sums in float64 and divides by the count.
"""

import numpy as np

# ---- problem constants (hardcoded; kernel.py must be self-contained) ----
B, C, D, H, W = 2, 1, 32, 128, 128
N_CORES = 8
D_CHUNK = D // 4  # 8 slices per core
SLAB_S = D_CHUNK + 2  # with halo
SLAB_W = W + 2  # with W zero-pad
N_OFFSETS = 27
TOTAL_COUNT = N_OFFSETS * B * C * D * H * W

_cached = None


def _build():
    """Build and schedule the Bass program once; return (nc, out_name)."""
    import concourse.bass as bass
    import concourse.tile as tile
    from concourse import bacc, mybir

    f32 = mybir.dt.float32
    nc = bacc.Bacc(
        "TRN2",
        target_bir_lowering=False,
        debug=False,
        enable_asserts=False,
        num_devices=N_CORES,
    )
    xs = nc.dram_tensor("xs", [H, SLAB_S, SLAB_W], f32, kind="ExternalInput").ap()
    ys = nc.dram_tensor("ys", [H, SLAB_S, SLAB_W], f32, kind="ExternalInput").ap()
    out = nc.dram_tensor("out", [H, 52], f32, kind="ExternalOutput").ap()

    with tile.TileContext(nc) as tc:
        with tc.tile_pool(name="main", bufs=1) as pool, tc.tile_pool(
            name="scrap", bufs=4
        ) as scrap_pool:
            xt = pool.tile([H, SLAB_S, SLAB_W], f32)
            yt = pool.tile([H, SLAB_S, SLAB_W], f32)
            nc.sync.dma_start(xt[:], xs[:])
            nc.sync.dma_start(yt[:], ys[:])

            d0 = pool.tile([H, SLAB_S, SLAB_W], f32)  # kh = 1 (no H shift)
            nc.vector.tensor_tensor(
                out=d0[:], in0=xt[:], in1=yt[:], op=mybir.AluOpType.subtract
            )

            # H-shifted copies: dm[h] = d0[h-1], dp[h] = d0[h+1]
            dm = pool.tile([H, SLAB_S, SLAB_W], f32)
            dp = pool.tile([H, SLAB_S, SLAB_W], f32)
            nc.gpsimd.memset(dm[:], 0.0)
            nc.gpsimd.memset(dp[:], 0.0)
            nc.sync.dma_start(dm[1:H], d0[0 : H - 1])
            nc.sync.dma_start(dp[0 : H - 1], d0[1:H])

            acc = pool.tile([H, 52], f32)
            bufs = {0: dm, 1: d0, 2: dp}
            center = d0[:, 1 : 1 + D_CHUNK, 1 : 1 + W]
            o = 0
            for kd in range(3):
                for kh in range(3):
                    for kw in range(3):
                        if kd == 1 and kh == 1 and kw == 1:
                            continue
                        shifted = bufs[kh][:, kd : kd + D_CHUNK, kw : kw + W]
                        for col, op1 in (
                            (2 * o, mybir.AluOpType.max),
                            (2 * o + 1, mybir.AluOpType.min),
                        ):
                            sc = scrap_pool.tile([H, D_CHUNK, W], f32, tag="sc")
                            nc.vector.scalar_tensor_tensor(
                                out=sc[:],
                                in0=center,
                                scalar=1.0,
                                in1=shifted,
                                op0=mybir.AluOpType.mult,
                                op1=op1,
                                accum_out=acc[:, col : col + 1],
                            )
                        o += 1
            nc.sync.dma_start(out[:], acc[:])

    nc.compile()
    return nc, "out"


def _make_slab(full: np.ndarray, b: int, d0: int) -> np.ndarray:
    """[H, SLAB_S, SLAB_W] fp32 slab with halo slices and W pad, zeros outside."""
    slab = np.zeros((H, SLAB_S, SLAB_W), dtype=np.float32)
    lo, hi = d0 - 1, d0 + D_CHUNK + 1
    clo, chi = max(lo, 0), min(hi, D)
    # full[b,0,d,h,w] -> slab[h, d-lo, 1+w]
    chunk = full[b, 0, clo:chi]  # [n, H, W]
    slab[:, clo - lo : chi - lo, 1 : 1 + W] = np.transpose(chunk, (1, 0, 2))
    return slab


def kernel(x: np.ndarray, y: np.ndarray) -> np.ndarray:
    global _cached
    if _cached is None:
        _cached = _build()
    nc, out_name = _cached

    from concourse.bass_utils import run_bass_kernel_spmd

    x = np.asarray(x, dtype=np.float32)
    y = np.asarray(y, dtype=np.float32)
    in_maps = []
    for core in range(N_CORES):
        b, chunk = divmod(core, 4)
        d0 = chunk * D_CHUNK
        in_maps.append({"xs": _make_slab(x, b, d0), "ys": _make_slab(y, b, d0)})

    res = run_bass_kernel_spmd(nc, in_maps, core_ids=list(range(N_CORES)))

    total = np.float64(0.0)
    for core in range(N_CORES):
        a = res.results[core][out_name].astype(np.float64)
        total += a[:, 0::2].sum() - a[:, 1::2].sum()
    return np.asarray(np.float64(total) / TOTAL_COUNT, dtype=np.float32)


# revision 8
# speedup vs baseline: 1.2396x; 1.2396x over previous
"""Central-difference L1 loss kernel for 8 trn2 NeuronCores.

Math: with d = x - y, the loss is
    mean_{27 offsets o} |d[v] - d_pad[v + o]|
over the (B,C,D,H,W) = (2,1,32,128,128) volume, zero-padded by 1 in D/H/W.
(The x_diff - y_diff of the reference collapses to differences of d.)

Sharding: 8 shards over (B=2) x (D in 4 chunks of 8 slices). Each core gets a
[128(H), 10(slices incl halo), 130(W incl pad)] fp32 slab of x and y with
zeros in the halo/pad positions that fall outside the volume.

Device per core:
  d = xs - ys                              (DVE subtract)
  dm[h] = d[h-1], dp[h] = d[h+1]           (SBUF->SBUF DMA partition shift)
  for each of 26 nonzero offsets:          (center offset contributes 0)
      acc[2o]   = sum over owned voxels of max(center, shifted)
      acc[2o+1] = sum over owned voxels of min(center, shifted)
  (sum|a-b| = sum max - sum min; DVE tensor_tensor_reduce has no abs op)
Host folds the 8x[128,52] partial sums in float64 and divides by the count.
"""

import numpy as np

# ---- problem constants (hardcoded; kernel.py must be self-contained) ----
B, C, D, H, W = 2, 1, 32, 128, 128
N_CORES = 8
D_CHUNK = D // 4  # 8 slices per core
SLAB_S = D_CHUNK + 2  # with halo
SLAB_W = W + 2  # with W zero-pad
N_OFFSETS = 27
TOTAL_COUNT = N_OFFSETS * B * C * D * H * W

_cached = None
_ABS_OP = None


def _register_abs_diff_op():
    """Register a custom DVE op: out=|in0-in1|, accum_out=sum(out). One DVE
    pass per offset instead of the native max+min two-pass."""
    global _ABS_OP
    if _ABS_OP is not None:
        return _ABS_OP
    import numpy as np
    from operator import add

    import concourse.dve_ops as dve_ops
    from concourse.dve_ops import OPS, CUSTOM_DVE_SPECS, DveOp
    from concourse.dve_spec import Spec, Src0, Src1, lower, maxx
    from concourse.dve_uop import DveOpSpec

    name = "ABS_DIFF_ACC_ANT"
    if name in dve_ops._SUB_OPCODE_FOR_NAME:
        _ABS_OP = next(op for op in OPS if op.name == name)
        return _ABS_OP

    def _ref(in0, in1, s0, s1, imm2):
        b = np.abs(in0.astype(np.float32) - in1.astype(np.float32))
        return b, b.reshape(b.shape[0], -1).sum(axis=-1, keepdims=True)

    spec = Spec(body=maxx(Src0 - Src1, Src1 - Src0), accum=add, reference=_ref)
    row = max(dve_ops._SUB_OPCODE_FOR_NAME.values()) + 1
    assert row < 0x20
    dve_ops._SUB_OPCODE_FOR_NAME[name] = row
    shas = {}
    for ver in ("v3", "v4"):
        try:
            shas[ver] = DveOpSpec(
                name=name, opcode=row, uops=lower(spec, ver=ver), rd1_en=True
            ).sha(ver)
        except Exception:
            pass
    op = DveOp(name, spec, subdim=False, uops_sha=shas)
    OPS.append(op)
    CUSTOM_DVE_SPECS[name] = spec
    _ABS_OP = op
    return op


def _build():
    """Build and schedule the Bass program once; return (nc, out_name)."""
    import concourse.bass as bass
    import concourse.tile as tile
    from concourse import bacc, mybir

    abs_op = _register_abs_diff_op()
    f32 = mybir.dt.float32
    nc = bacc.Bacc(
        "TRN2",
        target_bir_lowering=False,
        debug=False,
        enable_asserts=False,
        num_devices=N_CORES,
    )
    xs = nc.dram_tensor("xs", [H, SLAB_S, SLAB_W], f32, kind="ExternalInput").ap()
    ys = nc.dram_tensor("ys", [H, SLAB_S, SLAB_W], f32, kind="ExternalInput").ap()
    out = nc.dram_tensor("out", [H, 26], f32, kind="ExternalOutput").ap()

    with tile.TileContext(nc) as tc:
        with tc.tile_pool(name="main", bufs=1) as pool, tc.tile_pool(
            name="scrap", bufs=4
        ) as scrap_pool:
            xt = pool.tile([H, SLAB_S, SLAB_W], f32)
            yt = pool.tile([H, SLAB_S, SLAB_W], f32)
            nc.sync.dma_start(xt[:], xs[:])
            nc.sync.dma_start(yt[:], ys[:])

            d0 = pool.tile([H, SLAB_S, SLAB_W], f32)  # kh = 1 (no H shift)
            nc.vector.tensor_tensor(
                out=d0[:], in0=xt[:], in1=yt[:], op=mybir.AluOpType.subtract
            )

            # H-shifted copies: dm[h] = d0[h-1], dp[h] = d0[h+1]
            dm = pool.tile([H, SLAB_S, SLAB_W], f32)
            dp = pool.tile([H, SLAB_S, SLAB_W], f32)
            nc.gpsimd.memset(dm[:], 0.0)
            nc.gpsimd.memset(dp[:], 0.0)
            nc.sync.dma_start(dm[1:H], d0[0 : H - 1])
            nc.sync.dma_start(dp[0 : H - 1], d0[1:H])

            acc = pool.tile([H, 26], f32)
            bufs = {0: dm, 1: d0, 2: dp}
            center = d0[:, 1 : 1 + D_CHUNK, 1 : 1 + W]
            o = 0
            for kd in range(3):
                for kh in range(3):
                    for kw in range(3):
                        if kd == 1 and kh == 1 and kw == 1:
                            continue
                        shifted = bufs[kh][:, kd : kd + D_CHUNK, kw : kw + W]
                        sc = scrap_pool.tile([H, D_CHUNK, W], f32, tag="sc")
                        nc.vector._custom_dve(
                            abs_op,
                            out=sc[:],
                            in0=center,
                            in1=shifted,
                            accum_out=acc[:, o : o + 1],
                        )
                        o += 1
            nc.sync.dma_start(out[:], acc[:])

    nc.compile()
    return nc, "out"


def _make_slab(full: np.ndarray, b: int, d0: int) -> np.ndarray:
    """[H, SLAB_S, SLAB_W] fp32 slab with halo slices and W pad, zeros outside."""
    slab = np.zeros((H, SLAB_S, SLAB_W), dtype=np.float32)
    lo, hi = d0 - 1, d0 + D_CHUNK + 1
    clo, chi = max(lo, 0), min(hi, D)
    # full[b,0,d,h,w] -> slab[h, d-lo, 1+w]
    chunk = full[b, 0, clo:chi]  # [n, H, W]
    slab[:, clo - lo : chi - lo, 1 : 1 + W] = np.transpose(chunk, (1, 0, 2))
    return slab


def kernel(x: np.ndarray, y: np.ndarray) -> np.ndarray:
    global _cached
    if _cached is None:
        _cached = _build()
    nc, out_name = _cached

    from concourse.bass_utils import run_bass_kernel_spmd

    x = np.asarray(x, dtype=np.float32)
    y = np.asarray(y, dtype=np.float32)
    in_maps = []
    for core in range(N_CORES):
        b, chunk = divmod(core, 4)
        d0 = chunk * D_CHUNK
        in_maps.append({"xs": _make_slab(x, b, d0), "ys": _make_slab(y, b, d0)})

    res = run_bass_kernel_spmd(nc, in_maps, core_ids=list(range(N_CORES)))

    total = np.float64(0.0)
    for core in range(N_CORES):
        total += res.results[core][out_name].astype(np.float64).sum()
    return np.asarray(np.float64(total) / TOTAL_COUNT, dtype=np.float32)


# revision 12
# speedup vs baseline: 2.2650x; 1.8272x over previous
"""Central-difference L1 loss kernel for 8 trn2 NeuronCores.

Math: with d = x - y, the loss is
    mean_{27 offsets o} |d[v] - d_pad[v + o]|
over the (B,C,D,H,W) = (2,1,32,128,128) volume, zero-padded by 1 in D/H/W.
(The x_diff - y_diff of the reference collapses to differences of d.)

Sharding: 8 shards over (B=2) x (D in 4 chunks of 8 slices). Each core gets a
[128(H), 10(slices incl halo), 130(W incl pad)] fp32 slab of x and y with
zeros in the halo/pad positions that fall outside the volume.

Device per core:
  d = xs - ys                              (DVE subtract)
  dm[h] = d[h-1], dp[h] = d[h+1]           (SBUF->SBUF DMA partition shift)
  for each of 26 nonzero offsets:          (center offset contributes 0)
      acc[2o]   = sum over owned voxels of max(center, shifted)
      acc[2o+1] = sum over owned voxels of min(center, shifted)
  (sum|a-b| = sum max - sum min; DVE tensor_tensor_reduce has no abs op)
Host folds the 8x[128,52] partial sums in float64 and divides by the count.
"""

import numpy as np

# ---- problem constants (hardcoded; kernel.py must be self-contained) ----
B, C, D, H, W = 2, 1, 32, 128, 128
N_CORES = 8
D_CHUNK = D // 4  # 8 slices per core
SLAB_S = D_CHUNK + 2  # with halo
SLAB_W = W + 2  # with W zero-pad
N_OFFSETS = 27
TOTAL_COUNT = N_OFFSETS * B * C * D * H * W

_cached = None
_ABS_OP = None


def _register_abs_diff_op():
    """Register a custom DVE op: out=|in0-in1|, accum_out=sum(out). One DVE
    pass per offset instead of the native max+min two-pass."""
    global _ABS_OP
    if _ABS_OP is not None:
        return _ABS_OP
    import numpy as np
    from operator import add

    import concourse.dve_ops as dve_ops
    from concourse.dve_ops import OPS, CUSTOM_DVE_SPECS, DveOp
    from concourse.dve_spec import Spec, Src0, Src1, lower, maxx
    from concourse.dve_uop import DveOpSpec

    name = "ABS_DIFF_ACC_ANT"
    if name in dve_ops._SUB_OPCODE_FOR_NAME:
        _ABS_OP = next(op for op in OPS if op.name == name)
        return _ABS_OP

    def _ref(in0, in1, s0, s1, imm2):
        b = np.abs(in0.astype(np.float32) - in1.astype(np.float32))
        return b, b.reshape(b.shape[0], -1).sum(axis=-1, keepdims=True)

    spec = Spec(body=maxx(Src0 - Src1, Src1 - Src0), accum=add, reference=_ref)
    row = max(dve_ops._SUB_OPCODE_FOR_NAME.values()) + 1
    assert row < 0x20
    dve_ops._SUB_OPCODE_FOR_NAME[name] = row
    shas = {}
    for ver in ("v3", "v4"):
        try:
            shas[ver] = DveOpSpec(
                name=name, opcode=row, uops=lower(spec, ver=ver), rd1_en=True
            ).sha(ver)
        except Exception:
            pass
    op = DveOp(name, spec, subdim=False, uops_sha=shas)
    OPS.append(op)
    CUSTOM_DVE_SPECS[name] = spec
    _ABS_OP = op
    return op


def _build():
    """Build and schedule the Bass program once; return (nc, out_name)."""
    import concourse.bass as bass
    import concourse.tile as tile
    from concourse import bacc, mybir

    abs_op = _register_abs_diff_op()
    f32 = mybir.dt.float32
    nc = bacc.Bacc(
        "TRN2",
        target_bir_lowering=False,
        debug=False,
        enable_asserts=False,
        num_devices=N_CORES,
    )
    xs = nc.dram_tensor("xs", [H, SLAB_S, SLAB_W], f32, kind="ExternalInput").ap()
    ys = nc.dram_tensor("ys", [H, SLAB_S, SLAB_W], f32, kind="ExternalInput").ap()
    shm = nc.dram_tensor("shm", [H, 2, H], f32, kind="ExternalInput").ap()
    out = nc.dram_tensor("out", [H, 26], f32, kind="ExternalOutput").ap()

    FLAT = SLAB_S * SLAB_W
    with tile.TileContext(nc) as tc:
        with (
            tc.tile_pool(name="main", bufs=1) as pool,
            tc.tile_pool(name="scrap", bufs=4) as scrap_pool,
            tc.tile_pool(name="psum", bufs=1, space="PSUM") as psum_pool,
        ):
            sh = pool.tile([H, 2, H], f32)
            nc.sync.dma_start(sh[:], shm[:])
            xt = pool.tile([H, SLAB_S, SLAB_W], f32)
            yt = pool.tile([H, SLAB_S, SLAB_W], f32)
            nc.sync.dma_start(xt[:], xs[:])
            nc.sync.dma_start(yt[:], ys[:])

            d0 = pool.tile([H, SLAB_S, SLAB_W], f32)  # kh = 1 (no H shift)
            nc.vector.tensor_tensor(
                out=d0[:], in0=xt[:], in1=yt[:], op=mybir.AluOpType.subtract
            )

            # H-shifted copies via TensorE: dm[h] = d0[h-1], dp[h] = d0[h+1]
            # (shift matrices give zero boundary rows for free)
            dmp = psum_pool.tile([H, SLAB_S, SLAB_W], f32)
            dpp = psum_pool.tile([H, SLAB_S, SLAB_W], f32)
            d0_flat = d0[:].rearrange("p a b -> p (a b)")
            chunks = [(c, min(c + 512, FLAT)) for c in range(0, FLAT, 512)]
            for dst, mi in ((dmp, 0), (dpp, 1)):
                dflat = dst[:].rearrange("p a b -> p (a b)")
                for c0, c1 in chunks:
                    nc.tensor.matmul(
                        dflat[:, c0:c1],
                        sh[:, mi, :],
                        d0_flat[:, c0:c1],
                        start=True,
                        stop=True,
                    )

            acc = pool.tile([H, 26], f32)
            bufs = {0: dmp, 1: d0, 2: dpp}
            center = d0[:, 1 : 1 + D_CHUNK, 1 : 1 + W]
            o = 0
            for kh in (1, 0, 2):  # kh=1 first: no dependency on the matmuls
                for kd in range(3):
                    for kw in range(3):
                        if kd == 1 and kh == 1 and kw == 1:
                            continue
                        shifted = bufs[kh][:, kd : kd + D_CHUNK, kw : kw + W]
                        sc = scrap_pool.tile([H, D_CHUNK, W], f32, tag="sc")
                        nc.vector._custom_dve(
                            abs_op,
                            out=sc[:],
                            in0=center,
                            in1=shifted,
                            accum_out=acc[:, o : o + 1],
                        )
                        o += 1
            nc.sync.dma_start(out[:], acc[:])

    nc.compile()
    return nc, "out"


def _make_slab(full: np.ndarray, b: int, d0: int) -> np.ndarray:
    """[H, SLAB_S, SLAB_W] fp32 slab with halo slices and W pad, zeros outside."""
    slab = np.zeros((H, SLAB_S, SLAB_W), dtype=np.float32)
    lo, hi = d0 - 1, d0 + D_CHUNK + 1
    clo, chi = max(lo, 0), min(hi, D)
    # full[b,0,d,h,w] -> slab[h, d-lo, 1+w]
    chunk = full[b, 0, clo:chi]  # [n, H, W]
    slab[:, clo - lo : chi - lo, 1 : 1 + W] = np.transpose(chunk, (1, 0, 2))
    return slab


def _make_in_maps(x: np.ndarray, y: np.ndarray) -> list:
    x = np.asarray(x, dtype=np.float32)
    y = np.asarray(y, dtype=np.float32)
    shm = np.stack(
        [np.eye(H, k=1, dtype=np.float32), np.eye(H, k=-1, dtype=np.float32)],
        axis=1,
    )  # [H, 2, H]; shm[:,0] shifts down (dm), shm[:,1] shifts up (dp)
    in_maps = []
    for core in range(N_CORES):
        b, chunk = divmod(core, 4)
        d0 = chunk * D_CHUNK
        in_maps.append(
            {"xs": _make_slab(x, b, d0), "ys": _make_slab(y, b, d0), "shm": shm}
        )
    return in_maps


def kernel(x: np.ndarray, y: np.ndarray) -> np.ndarray:
    global _cached
    if _cached is None:
        _cached = _build()
    nc, out_name = _cached

    from concourse.bass_utils import run_bass_kernel_spmd

    in_maps = _make_in_maps(x, y)

    res = run_bass_kernel_spmd(nc, in_maps, core_ids=list(range(N_CORES)))

    total = np.float64(0.0)
    for core in range(N_CORES):
        total += res.results[core][out_name].astype(np.float64).sum()
    return np.asarray(np.float64(total) / TOTAL_COUNT, dtype=np.float32)


# revision 37
# speedup vs baseline: 3.2580x; 1.4384x over previous
"""Central-difference L1 loss kernel for 8 trn2 NeuronCores.

Math: with d = x - y, the loss is
    mean_{27 offsets o} |d[v] - d_pad[v + o]|
over the (B,C,D,H,W) = (2,1,32,128,128) volume, zero-padded by 1 in D/H/W.
(The x_diff - y_diff of the reference collapses to differences of d.)

Sharding: 8 shards over (B=2) x (D in 4 chunks of 8 slices). Each core gets a
[128(H), 2(x|y), 10(slices incl halo), 132(W incl pad)] fp32 slab with zeros
in halo/pad positions that fall outside the volume.

Device per core (bf16 pipeline):
  d0  = x - y  (bf16; DVE)
  d0s = d0 shifted by one flat element (ACT copy; fixes bf16 pair alignment
        for the w+-1 offsets so the DVE runs its 2x_1p packed mode)
  dm/dp   = d0 shifted -/+1 along H (partition axis) via TensorE matmul with
            super/sub-diagonal shift matrices -> PSUM -> bf16 SBUF (ACT copy);
            the matmul gives zero boundary rows for free
  dms/dps = same from the w-shifted view
  For most offsets: one custom DVE op per offset:
      accum_out[p] = sum |center - shifted|   (hand-registered ABS_DIFF_ACC,
      with a hand-built 2x_1p uop program processing packed bf16 pairs)
  For a few offsets (PE_SET): TensorE computes t = center - shifted in PSUM
      (I-matmul + negated-shift-matmul accumulation), ScalarE abs-accumulates.
Host folds the 8x[128,26] partial sums in float64 and divides by the count.
"""

import numpy as np

# ---- problem constants (hardcoded; kernel.py must be self-contained) ----
B, C, D, H, W = 2, 1, 32, 128, 128
N_CORES = 8
D_CHUNK = D // 4  # 8 slices per core
SLAB_S = D_CHUNK + 2  # with halo
SLAB_W = W + 4  # W + 2 pad each side (keeps slice stride & data start even)
FLAT = SLAB_S * SLAB_W
N_OFFSETS = 27
TOTAL_COUNT = N_OFFSETS * B * C * D * H * W

# offsets computed on the TensorE+ScalarE pathway instead of the DVE
PE_SET = {(0, 0, 0), (2, 0, 2), (0, 2, 2), (2, 2, 0)}  # (kd, kh, kw)


def offset_order():
    """All 26 (kd, kh, kw) offsets, ordered so passes whose buffers are ready
    earliest come first. Returns [(kd, kh, kw, pathway)] with pathway in
    {"dve", "pe"}."""
    groups = [[], [], [], []]
    for kd in range(3):
        for kh in range(3):
            for kw in range(3):
                if kd == kh == kw == 1:
                    continue
                if kh == 1 and kw == 1:
                    g = 0  # needs d0 only
                elif kh == 1:
                    g = 1  # needs d0s
                elif kw == 1:
                    g = 2  # needs dm/dp
                else:
                    g = 3  # needs dms/dps (or PE)
                groups[g].append((kd, kh, kw))
    order = []
    for g in range(4):
        for kd, kh, kw in groups[g]:
            pw = "pe" if (kd, kh, kw) in PE_SET else "dve"
            order.append((kd, kh, kw, pw))
    return order


_cached = None
_ABS_OP = None


def _register_abs_diff_op():
    """Register two custom DVE op rows:
      ABS2X_SEED: seed (acc <- 0) + steady; ABS2X_CONT: steady only (the
    hardware accumulator keeps integrating across instructions).
    Steady body (both rows, both modes) uses the native v3 ABSOLUTE_DIFF op:
      1x: |a - b| per element; 2x: |a-b| of the packed lo+hi bf16 pair summed.
    Machine shape throughout: accumulate recurrence early (CURR_ALU_OUT), acc
    rides the BYPASS chain with a_flop re-latched on every block to the end;
    DVE_READ_ACCUMULATOR2 taps that chain. The read only decodes correctly
    when the op's dst dtype is fp32, so the hot bf16 passes skip accum_out and
    a final tiny fp32-dst flush op (in0 == in1, adds 0) extracts the total."""
    global _ABS_OP
    if _ABS_OP is not None:
        return _ABS_OP
    from dataclasses import dataclass
    from operator import add

    import concourse.dve_ops as dve_ops
    from concourse.dve_ops import OPS, CUSTOM_DVE_SPECS, DveOp
    from concourse.dve_spec import Spec, Src0, Src1, lower, maxx
    from concourse.dve_uop import (
        AluInp,
        AluOp,
        DelayInp,
        DveOpSpec,
        InpSel,
        OutPath,
        OutSel,
        Trigger,
        UopConfig,
        UopDpConfig,
    )

    def _ref(in0, in1, s0, s1, imm2):
        b = np.abs(in0.astype(np.float32) - in1.astype(np.float32))
        return b, b.reshape(b.shape[0], -1).sum(axis=-1, keepdims=True)

    spec = Spec(body=maxx(Src0 - Src1, Src1 - Src0), accum=add, reference=_ref)

    PA, CA = AluInp.PREV_ALU_OUT, AluInp.CURR_ALU_OUT
    PD = lambda n: AluInp(int(AluInp.PREV_DELAY_0) + n)

    def mk_uop(kind, two_x):
        INP = [
            InpSel.SRC_0,
            InpSel.SRC_1,
            InpSel.SRC_0_HI if two_x else InpSel.ZERO,
            InpSel.SRC_1_HI if two_x else InpSel.ZERO,
        ] + [InpSel.ZERO] * 4
        INP_EN = ([1, 1, 1, 1] if two_x else [1, 1, 0, 0]) + [0, 0, 0, 0]
        bs = []
        for _ in range(8):
            b = UopDpConfig()
            b.op, b.alu_src0, b.alu_src1 = AluOp.BYPASS, PA, PA
            b.alu_out_enable = 1
            bs.append(b)

        def alu(i, op, s0, s1):
            bs[i].op, bs[i].alu_src0, bs[i].alu_src1 = op, s0, s1

        def chain(i, n, src=DelayInp.PREV_DELAY):
            bs[i].delay[n] = src
            bs[i].delay_enable[n] = 1

        if kind == "seed":
            acc_stage = 3
            alu(3, AluOp.BITWISE_XOR, PA, PA)  # acc <- 0
        elif two_x:
            acc_stage = 3
            alu(0, AluOp.ABSOLUTE_DIFF, PA, PD(0))  # |a_lo - b_lo|
            alu(1, AluOp.ABSOLUTE_DIFF, PD(1), PD(2))  # |a_hi - b_hi|
            alu(2, AluOp.ADD, PA, PD(3))  # pair sum
            alu(3, AluOp.ADD, CA, PA)  # accumulate
            chain(0, 1)  # a_hi to blk1
            chain(0, 2)  # b_hi to blk1
            chain(1, 3, DelayInp.PREV_ALU_OUT)  # chain3 <- |d_lo|
            chain(3, 0, DelayInp.PREV_ALU_OUT)  # chain0 <- body (for out)
            for i in (4, 5, 6, 7):
                chain(i, 0)
        else:
            # accum stage MUST match the 2x program (block 3): the running
            # total lives in that block's out-flop across chained ops, and a
            # mode-mismatched op in the chain must find it in the same place
            acc_stage = 3
            alu(0, AluOp.ABSOLUTE_DIFF, PA, PD(0))  # |a - b|
            alu(3, AluOp.ADD, CA, PA)  # accumulate
            chain(3, 0, DelayInp.PREV_ALU_OUT)  # chain0 <- body (for out)
            for i in (4, 5, 6, 7):
                chain(i, 0)
        for i in range(acc_stage, 8):
            bs[i].alu_out_a_enable = 1
        u = UopConfig(
            datapath_config=bs,
            inp=list(INP),
            inp_enable=list(INP_EN),
            accum_enabled=1,
            require_inp0=0 if kind == "seed" else 1,
            require_inp1=0 if kind == "seed" else 1,
            trigger=(
                (Trigger.COUNT, Trigger.NONE, Trigger.NONE)
                if kind == "seed"
                else (Trigger.SRC_TENSOR_DONE, Trigger.NONE, Trigger.NONE)
            ),
            next_uop=(1, 0, 0) if kind == "seed" else (0, 0, 0),
            repeat_count=1 if kind == "seed" else 0,
        )
        if kind != "seed":
            u.out[OutPath.WR0_LO] = OutSel.DELAY_0
            u.out_enable[OutPath.WR0_LO] = 1
            if two_x:
                u.out[OutPath.WR0_HI] = OutSel.DELAY_0
                u.out_enable[OutPath.WR0_HI] = 1
        return u

    def register(name, with_seed):
        row = max(dve_ops._SUB_OPCODE_FOR_NAME.values()) + 1
        assert row < 0x20
        dve_ops._SUB_OPCODE_FOR_NAME[name] = row

        if with_seed:
            u1 = [mk_uop("seed", False), mk_uop("steady", False)]
            u2 = [mk_uop("seed", True), mk_uop("steady", True)]
        else:
            u1 = [mk_uop("steady", False)]
            u2 = [mk_uop("steady", True)]

        @dataclass(frozen=True)
        class DveOpHand(DveOp):
            def compile(self, ver):
                key = (self.name, ver)
                if (r := dve_ops._COMPILE_CACHE.get(key)) is not None:
                    return r
                if ver == "v3":
                    r = DveOpSpec(
                        name=self.name, opcode=row, uops=u1, uops_2x=u2,
                        rd1_en=True, perf_max=1,
                    )
                else:
                    r = DveOpSpec(
                        name=self.name, opcode=row,
                        uops=lower(spec, ver=ver), rd1_en=True,
                    )
                dve_ops._COMPILE_CACHE[key] = r
                return r

        op = DveOpHand(name, spec, subdim=False, uops_sha={})
        OPS.append(op)
        CUSTOM_DVE_SPECS[name] = spec
        return op

    _ABS_OP = (register("ABS2X_SEED_V7_ANT", True), register("ABS2X_CONT_V7_ANT", False))
    return _ABS_OP


def _emit_abs(nc, op, out, in0, in1, accum_out=None, s0=0.0):
    """_custom_dve clone that sets perf_max=1 (byte-36[7:6]) so the engine
    picks the 2x_1p uop slot when the APs qualify (silent 1x fallback)."""
    import concourse.bass_isa as bass_isa
    from concourse import mybir
    from concourse.dve_ops import get_dve_sub_opcode

    v = nc.vector
    if op.name not in nc.m.ant_custom_dve_ops:
        nc.m.ant_custom_dve_ops = sorted({*nc.m.ant_custom_dve_ops, op.name})
    shape = bass_isa.CustomDveShape.STT
    isa_opcode = nc.isa.Opcode[
        f"NEURON_ISA_TPB_OPCODE_CUSTOM_DVE_ANT_{shape.slot()}"
    ].value
    zero = mybir.ImmediateValue(dtype=mybir.dt.float32, value=0.0)
    s0_l = v.lower_ap(s0, for_isa=True) if not isinstance(s0, float) else zero
    ins = [
        v.lower_ap(in0, for_isa=True, opt=True),
        v.lower_ap(in1, for_isa=True, opt=True),
        s0_l,
        zero,
    ]
    outs = [v.lower_ap(out, for_isa=True, opt=True)]
    if accum_out is not None:
        outs.append(v.lower_ap(accum_out, for_isa=True))
    return v.add_instruction(
        bass_isa.InstCustomDveAnt(
            name=nc.get_next_instruction_name(),
            op_name=op.name,
            rd1_en=True,
            subdim=0,
            imm2=0.0,
            shape=shape,
            row=get_dve_sub_opcode(op.name),
            isa_opcode=isa_opcode,
            ins=ins,
            outs=outs,
            perf_max=1,
        )
    )


def _build():
    """Build and schedule the Bass program once; return (nc, out_name)."""
    import concourse.tile as tile
    from concourse import bacc, mybir

    seed_op, cont_op = _register_abs_diff_op()
    f32 = mybir.dt.float32
    bf16 = mybir.dt.bfloat16
    AF = mybir.ActivationFunctionType
    nc = bacc.Bacc(
        "TRN2",
        target_bir_lowering=False,
        debug=False,
        enable_asserts=False,
        num_devices=N_CORES,
    )
    xy = nc.dram_tensor("xy", [H, 2, SLAB_S, SLAB_W], f32, kind="ExternalInput").ap()
    shm = nc.dram_tensor("shm", [H, 5, H], bf16, kind="ExternalInput").ap()
    out = nc.dram_tensor("out", [H, 8], f32, kind="ExternalOutput").ap()

    order = offset_order()

    with tile.TileContext(nc) as tc:
        with (
            tc.tile_pool(name="main", bufs=1) as pool,
            tc.tile_pool(name="scrap", bufs=4) as scrap_pool,
            tc.tile_pool(name="psum", bufs=2, space="PSUM") as psum_pool,
        ):
            sh = pool.tile([H, 5, H], bf16)
            nc.sync.dma_start(sh[:], shm[:])
            xyt = pool.tile([H, 2, SLAB_S, SLAB_W], f32)
            d0 = pool.tile([H, SLAB_S, SLAB_W], bf16)
            half = SLAB_S // 2
            for s0, s1 in ((0, half), (half, SLAB_S)):
                nc.sync.dma_start(xyt[:, :, s0:s1], xy[:, :, s0:s1])
                nc.vector.tensor_tensor(
                    out=d0[:, s0:s1],
                    in0=xyt[:, 0, s0:s1],
                    in1=xyt[:, 1, s0:s1],
                    op=mybir.AluOpType.subtract,
                )
            d0f = d0[:].rearrange("p a b -> p (a b)")

            # w-shifted twin (for w-odd offsets on the DVE path)
            d0s = pool.tile([H, SLAB_S, SLAB_W], bf16)
            d0sf = d0s[:].rearrange("p a b -> p (a b)")
            nc.scalar.copy(d0sf[:, 0 : FLAT - 1], d0f[:, 1:FLAT])

            # H-shifted buffers via TensorE (zero boundary rows for free),
            # PSUM -> SBUF bf16 casts on ScalarE
            sbufs = {}
            for key, mi, ws in (
                ("dm", 0, 0),
                ("dp", 1, 0),
                ("dms", 0, 1),
                ("dps", 1, 1),
            ):
                ps = psum_pool.tile([H, FLAT], f32, tag="ps")
                for c0 in range(0, FLAT, 512):
                    c1 = min(c0 + 512, FLAT)
                    r1 = min(c1 + ws, FLAT)
                    nc.tensor.matmul(
                        ps[:, c0 : c0 + (r1 - c0 - ws)],
                        sh[:, mi, :],
                        d0f[:, c0 + ws : r1],
                        start=True,
                        stop=True,
                    )
                t = pool.tile([H, SLAB_S, SLAB_W], bf16, name=key)
                tf = t[:].rearrange("p a b -> p (a b)")
                n = FLAT - ws
                nc.scalar.copy(tf[:, 0:n], ps[:, 0:n])
                sbufs[key] = t

            primary = {0: sbufs["dm"], 1: d0, 2: sbufs["dp"]}
            twins = {0: sbufs["dms"], 1: d0s, 2: sbufs["dps"]}

            acc = pool.tile([H, 32], f32)
            dve_sc = pool.tile([H, D_CHUNK, W], bf16)  # shared scrap: WAW
            # dependencies keep the accumulator-chained DVE ops in order
            n_dve = 0
            n_pe = 0
            center = d0[:, 1 : 1 + D_CHUNK, 2 : 2 + W]
            for o, (kd, kh, kw, pw) in enumerate(order):
                if pw == "dve":
                    if kw == 1:
                        shifted = primary[kh][:, kd : kd + D_CHUNK, 2 : 2 + W]
                    elif kw == 0:
                        shifted = twins[kh][:, kd : kd + D_CHUNK, 0:W]
                    else:
                        shifted = twins[kh][:, kd : kd + D_CHUNK, 2 : 2 + W]
                    op = seed_op if n_dve == 0 else cont_op
                    _emit_abs(nc, op, dve_sc[:], center, shifted)
                    n_dve += 1
                else:
                    a_out = acc[:, n_pe : n_pe + 1]
                    n_pe += 1
                    ps_t = psum_pool.tile([H, 2, 512], f32, tag="ps")
                    neg = {0: 3, 2: 4}[kh]  # -S1 / -S2 column in shm
                    for j in (0, 1):
                        nc.tensor.matmul(
                            ps_t[:, j],
                            sh[:, 2, :],
                            d0[:, 1 + 4 * j : 5 + 4 * j, 2 : 2 + W],
                            start=True,
                            stop=False,
                        )
                        nc.tensor.matmul(
                            ps_t[:, j],
                            sh[:, neg, :],
                            d0[:, kd + 4 * j : kd + 4 * j + 4, kw + 1 : kw + 1 + W],
                            start=False,
                            stop=True,
                        )
                    sc = scrap_pool.tile([H, 2, 512], bf16, tag="sc")
                    nc.scalar.activation(
                        sc[:], ps_t[:], AF.Abs, accum_out=a_out
                    )
            # flush: tiny fp32-dst continue op; in0 == in1 adds 0; its
            # appended accumulator read decodes correctly (fp32) and lands
            # the grand total of all chained DVE passes in acc[:, 26]
            fl = pool.tile([H, 1, 2], f32)
            dummy = dve_sc[:, 0:1, 0:2]  # RAW dep: runs after the whole chain
            _emit_abs(nc, cont_op, fl[:], dummy, dummy, acc[:, n_pe : n_pe + 1])
            nc.sync.dma_start(out[:, 0 : n_pe + 1], acc[:, 0 : n_pe + 1])

    nc.compile()
    return nc, "out"


def _make_slab(x: np.ndarray, y: np.ndarray, b: int, d0: int) -> np.ndarray:
    """[H, 2, SLAB_S, SLAB_W] fp32 slab (x|y) with halo slices and W pad."""
    slab = np.zeros((H, 2, SLAB_S, SLAB_W), dtype=np.float32)
    lo, hi = d0 - 1, d0 + D_CHUNK + 1
    clo, chi = max(lo, 0), min(hi, D)
    for t, full in ((0, x), (1, y)):
        chunk = full[b, 0, clo:chi]  # [n, H, W]
        slab[:, t, clo - lo : chi - lo, 2 : 2 + W] = np.transpose(chunk, (1, 0, 2))
    return slab


def _make_in_maps(x: np.ndarray, y: np.ndarray) -> list:
    import ml_dtypes

    x = np.asarray(x, dtype=np.float32)
    y = np.asarray(y, dtype=np.float32)
    eye = np.eye(H, dtype=np.float32)
    s1 = np.eye(H, k=1, dtype=np.float32)  # dm[p] = d0[p-1]
    s2 = np.eye(H, k=-1, dtype=np.float32)  # dp[p] = d0[p+1]
    shm = np.stack([s1, s2, eye, -s1, -s2], axis=1).astype(ml_dtypes.bfloat16)
    in_maps = []
    for core in range(N_CORES):
        b, chunk = divmod(core, 4)
        d0 = chunk * D_CHUNK
        in_maps.append({"xy": _make_slab(x, y, b, d0), "shm": shm})
    return in_maps


def kernel(x: np.ndarray, y: np.ndarray) -> np.ndarray:
    global _cached
    if _cached is None:
        _cached = _build()
    nc, out_name = _cached

    from concourse.bass_utils import run_bass_kernel_spmd

    in_maps = _make_in_maps(x, y)
    res = run_bass_kernel_spmd(nc, in_maps, core_ids=list(range(N_CORES)))

    total = np.float64(0.0)
    for core in range(N_CORES):
        total += res.results[core][out_name][:, :5].astype(np.float64).sum()
    return np.asarray(np.float64(total) / TOTAL_COUNT, dtype=np.float32)
